# revision 64
# baseline (speedup 1.0000x reference)
"""Causal multi-head attention on 8 Trainium2 NeuronCores — v3.

Problem: residual_stream [4, 2048, 1024] fp32, per-head QKV weights
[16, 1024, 64], output projection [1024, 1024]; causal softmax attention.

Sharding: tensor-parallel over heads — core c computes heads (2c, 2c+1)
for all 4 batches, applies the matching 128-row slice of weight_out, and
returns a full-shape fp16 partial output; the host sums the 8 partials.

v3 changes vs v2 (344us):
  * Startup: warmup matmuls run on a memset tile (no DMA dependency),
    weights DMA'd per-projection so the first QKV matmul starts as soon
    as w_q + xt chunk 0 land (v2 waited ~21us for the full weight+xt
    race to drain).
  * Normalize chain shortened: V_aug ones column holds 0.25 so the PV
    ones-row accumulates sum/4 directly; 4/sum comes from a single-pass
    reciprocal_approx_fast straight out of PSUM (v2: two tensor_scalar
    evacuations + DMA gather + full-rate reciprocal).  One broadcast
    multiply [128,512] instead of two [64,512].
  * Uniform 1-phase wout deferral (norm of chunk c early in chunk c+1's
    fill, wout late in the same fill), instead of v2's 2-phase chains
    that piled 6 wout groups into batch 3.
  * Batch 3 runs its chunks in reverse (3,2,1,0) so the serial tail is
    the smallest chunk; tail wout casts split ACT/DVE; V transposes for
    batch 3 complete during batch 2.
"""
import sys
import types

sys.path.insert(0, "/opt/trn_rl_repo")

import ml_dtypes
import numpy as np

import concourse.bass as bass
import concourse.tile as tile
from concourse import mybir

F32 = mybir.dt.float32
F16 = mybir.dt.float16

B = 4
SEQ = 2048
DM = 1024
DH = 64
NH = 16
NCORES = 8
HPC = NH // NCORES          # heads per core = 2
MT = DM // 128              # m-tiles = 8
KT = SEQ // 128             # k-tiles = 16
QC = SEQ // 512             # q-chunks of 512 = 4
EXP_BIAS = -6.0             # exp(s/8 - 6): softmax-shift, keeps P in fp16
OSC = 0.25                  # O evacuated as O/4; ones col = 1/4 so the
                            # sums row holds sum/4 and recip gives 4/sum

_CACHE = {}


def _split_waits(d, max_waits=1):
    # This walrus build allows a single sync-wait on several instruction
    # encodings (CTRL Drain, fused-LDW f32 Matmult). Hoist excess waits
    # onto same-engine NoOp carriers directly in the BIR JSON.
    for fn in d.get("functions", []):
        for blk in fn.get("blocks", []):
            out = []
            for inst in blk.get("instructions", []):
                si = inst.get("sync_info") or {}
                waits = si.get("on_wait") or []
                if len(waits) > max_waits:
                    extra = waits[: len(waits) - max_waits]
                    rest = waits[len(waits) - max_waits:]
                    for i, w in enumerate(extra):
                        out.append({
                            "name": f"{inst['name']}_sw{i}",
                            "opcode": "NoOp",
                            "engine": inst["engine"],
                            "ins": [],
                            "outs": [],
                            "sync_info": {"on_update": [], "on_wait": [w]},
                        })
                    inst = dict(inst)
                    inst["sync_info"] = {
                        "on_update": list(si.get("on_update") or []),
                        "on_wait": rest,
                    }
                out.append(inst)
            blk["instructions"] = out
    return d


def _patch_nc(nc):
    import orjson

    def to_json_bytes(self):
        return orjson.dumps(
            _split_waits(orjson.loads(mybir.module_to_json_bytes(self.m)))
        )

    nc.to_json_bytes = types.MethodType(to_json_bytes, nc)
    return nc


def _build_nc():
    nc = bass.Bass()

    xt = nc.dram_tensor("xt", [B, 128, QC, MT, 512], F16,
                    kind="ExternalInput")
    w_all = nc.dram_tensor("w_all", [128, MT, 3, 128], F16,
                           kind="ExternalInput")
    wout = nc.dram_tensor("wout", [128, DM], F16, kind="ExternalInput")
    tri = nc.dram_tensor("tri", [128, 128], F16, kind="ExternalInput")
    ident = nc.dram_tensor("ident", [128, 128], F16, kind="ExternalInput")
    y = nc.dram_tensor("y", [B, SEQ, DM], F16, kind="ExternalOutput")
    rrec = nc.dram_tensor("rrec", [B, QC, 1024], F16)   # 4/sum bounce

    with tile.TileContext(nc) as tc:
        with (
            tc.tile_pool(name="const", bufs=1) as const,
            tc.tile_pool(name="xtp", bufs=2) as xtp,
            tc.tile_pool(name="qp", bufs=2) as qp,
            tc.tile_pool(name="kp", bufs=2) as kp,
            tc.tile_pool(name="vp", bufs=2) as vp,
            tc.tile_pool(name="ptp", bufs=8) as ptp,
            tc.tile_pool(name="onp", bufs=2) as onp,
            tc.tile_pool(name="ysp", bufs=6) as ysp,
            tc.tile_pool(name="scp", bufs=2) as scp,
            tc.tile_pool(name="bcp", bufs=2) as bcp,
            tc.tile_pool(name="pss", bufs=2, space="PSUM") as pss,
            tc.tile_pool(name="pso", bufs=2, space="PSUM") as pso,
            tc.tile_pool(name="psf", bufs=2, space="PSUM") as psf,
        ):
            wrm = const.tile([128, 512], F16, tag="wrm")
            nc.gpsimd.memset(wrm[:], 0.001)
            dumt = const.tile([1, 1], F16, tag="dumt")

            w_t = const.tile([128, MT, 3, 128], F16, tag="w")
            nc.scalar.dma_start(out=w_t[:, 0:4], in_=w_all[:, 0:4])
            nc.scalar.dma_start(out=w_t[:, 4:8], in_=w_all[:, 4:8])
            wout_t = const.tile([128, DM], F16, tag="wout")
            nc.scalar.dma_start(out=wout_t, in_=wout[:])
            tri_t = const.tile([128, 128], F16, tag="tri")
            nc.scalar.dma_start(out=tri_t, in_=tri[:])
            ident_t = const.tile([128, 128], F16, tag="ident")
            nc.scalar.dma_start(out=ident_t, in_=ident[:])
            bias_t = const.tile([128, 1], F32, tag="bias")
            nc.gpsimd.memset(bias_t[:], EXP_BIAS)

            # V_aug double-buffered (batch parity); ones columns hold 1/4
            # so the PV ones-row accumulates sum/4.
            vaug0 = const.tile([128, KT, HPC, 65], F16, tag="vaug0")
            vaug1 = const.tile([128, KT, HPC, 65], F16, tag="vaug1")
            vaugs = [vaug0, vaug1]
            nc.gpsimd.memset(vaug0[:, :, :, 64:65], OSC)
            nc.gpsimd.memset(vaug1[:, :, :, 64:65], OSC)

            warm = psf.tile([128, 512], F32, tag="bank", name="warm")
            for i in range(24):
                nc.tensor.matmul(
                    warm[:],
                    wrm[:, 0:128],
                    wrm[:],
                    start=True, stop=True,
                )

            xts = {}

            def load_xt(b, eng_all=None):
                xt_t = xtp.tile([128, QC, MT, 512], F16, tag="xt",
                                name=f"xt{b}")
                xts[b] = xt_t
                for cq in range(QC):
                    if b == 0 and cq == 0:
                        # first chunk split across both queues: the very
                        # first projection only waits for half the bytes
                        nc.sync.dma_start(out=xt_t[:, 0, 0:4],
                                          in_=xt[b, :, 0, 0:4])
                        nc.gpsimd.dma_start(out=xt_t[:, 0, 4:8],
                                            in_=xt[b, :, 0, 4:8])
                        continue
                    eng = eng_all or (nc.gpsimd if cq % 2 else nc.sync)
                    eng.dma_start(out=xt_t[:, cq], in_=xt[b, :, cq])

            def gen_proj(b, out):
                """QKV projections for batch b as PE thunks (fine-grained:
                one thunk per (chunk, proj, m-half)). xts[b] is looked up
                lazily so load_xt(b) may be scheduled after gen time."""
                qt = qp.tile([128, SEQ], F16, tag="qt", name=f"qt{b}")
                kt_t = kp.tile([128, SEQ], F16, tag="kt", name=f"kt{b}")
                vt = vp.tile([128, SEQ], F16, tag="vt", name=f"vt{b}")
                out["qt"], out["kt"], out["vt"] = qt, kt_t, vt
                state = {}
                thunks = []

                def step(cq, proj, half):
                    xt_t = xts[b]
                    cqs = slice(cq * 512, cq * 512 + 512)
                    if half == 0:
                        state["ps"] = psf.tile([128, 512], F32, tag="bank",
                                               name=f"pj{b}_{cq}_{proj}")
                    ps = state["ps"]
                    for m in range(half * 4, half * 4 + 4):
                        nc.tensor.matmul(
                            ps[:],
                            w_t[:, m, proj, :],
                            xt_t[:, cq, m, :],
                            start=(m == 0),
                            stop=(m == MT - 1),
                        )
                    if half == 1:
                        if proj == 0:
                            nc.vector.tensor_copy(qt[:, cqs], ps[:])
                        elif proj == 1:
                            nc.vector.tensor_copy(kt_t[:, cqs], ps[:])
                        else:
                            nc.vector.tensor_copy(vt[:, cqs], ps[:])

                for cq in range(QC):
                    for proj in range(3):
                        for half in range(2):
                            thunks.append(
                                (2048,
                                 lambda cq=cq, proj=proj, half=half:
                                 step(cq, proj, half)))
                return thunks

            def gen_vtrans(b, vt, vaug, tks):
                """V^T -> V_aug transposes as PE thunks for given k-tiles."""
                def tstep(tk):
                    pt_ps = psf.tile([128, 128], F16, tag="bank",
                                     name=f"tp{b}_{tk}")
                    nc.tensor.transpose(
                        pt_ps[:], vt[:, tk * 128:(tk + 1) * 128], ident_t[:]
                    )
                    nc.vector.tensor_copy(vaug[:, tk, :, 0:64], pt_ps[:])
                return [(128, lambda tk=tk: tstep(tk)) for tk in tks]

            def gen_attention(b, cq, qt, kt_t, onorm, vaug):
                """Attention thunks for one (batch, chunk): one per k-tile
                plus trailing pipeline flush + chunk-finish."""
                ntk = 4 * cq + 4
                st = {"ops": None, "pend": []}
                cq0 = cq * 512

                def make_pt(tk):
                    j = tk - 4 * cq
                    off = 128 * j if j > 0 else 0
                    sdbl = pss.tile([128, 1024], F32, tag="bank",
                                    name=f"s{b}_{cq}_{tk}")
                    tks = slice(tk * 128, tk * 128 + 128)
                    for h in (0, 1):
                        hs = slice(h * 64, h * 64 + 64)
                        nc.tensor.matmul(
                            sdbl[:, h * 512 + off:(h + 1) * 512],
                            kt_t[hs, tks],
                            qt[hs, cq0 + off:cq0 + 512],
                            start=True, stop=True,
                        )
                    pt = ptp.tile([128, 1024], F16, tag="pt",
                                  name=f"pt{b}_{cq}_{tk}")
                    if j <= 0:
                        nc.scalar.activation(
                            pt[:], sdbl[:],
                            mybir.ActivationFunctionType.Exp,
                            bias=bias_t[:], scale=0.125,
                        )
                    else:
                        sr = sdbl[:].rearrange("p (h q) -> p h q",
                                               h=2)[:, :, off:512]
                        pr = pt[:].rearrange("p (h q) -> p h q",
                                             h=2)[:, :, off:512]
                        nc.scalar.activation(
                            pr, sr,
                            mybir.ActivationFunctionType.Exp,
                            bias=bias_t[:], scale=0.125,
                        )
                    if j >= 0:
                        pm = pt[:].rearrange(
                            "p (h q) -> p h q",
                            h=2)[:, :, 128 * j:128 * j + 128]
                        ta = tri_t[:]
                        tb = bass.AP(tensor=ta.tensor, offset=ta.offset,
                                     ap=[ta.ap[0], [0, 2], ta.ap[1]])
                        nc.vector.tensor_mul(pm, pm, tb)
                    return off, pt

                def pv_step(tk, off, pt):
                    for h in (0, 1):
                        nc.tensor.matmul(
                            st["ops"][h][:, off:512],
                            vaug[:, tk, h, :],
                            pt[:, h * 512 + off:(h + 1) * 512],
                            start=(tk == 0), stop=(tk == ntk - 1),
                        )

                def step(tk):
                    if tk == 0:
                        st["ops"] = {
                            h: pso.tile([65, 512], F32, tag="bank",
                                        name=f"o{b}_{cq}_{h}")
                            for h in (0, 1)
                        }
                    st["pend"].append((tk, make_pt(tk)))
                    if len(st["pend"]) > 2:
                        t0, (off0, p0) = st["pend"].pop(0)
                        pv_step(t0, off0, p0)

                def flush():
                    while st["pend"]:
                        t0, (off0, p0) = st["pend"].pop(0)
                        pv_step(t0, off0, p0)

                def finish():
                    cqs = slice(cq0, cq0 + 512)
                    ops = st["ops"]
                    # O/4 -> onorm fp16 (h0 on ACT, h1 on DVE); sums rows
                    # (already sum/4 via the 0.25 ones col) evacuate on ACT,
                    # then DMA-gather to [128,8] so the reciprocal runs wide
                    # (a [1,512] DVE op is one serial lane, ~3.3us).
                    nc.scalar.mul(onorm[0:64, cqs], ops[0][0:64, :], OSC)
                    nc.vector.tensor_scalar_mul(
                        onorm[64:128, cqs], ops[1][0:64, :], OSC)
                    scr = scp.tile([1, 1024], F32, tag="scr",
                                   name=f"scr{b}_{cq}")
                    nc.vector.tensor_copy(scr[:, 0:512], ops[0][64:65, :])
                    nc.vector.tensor_copy(scr[:, 512:1024],
                                          ops[1][64:65, :])
                    sc2 = scp.tile([128, 16], F32, tag="sc2",
                                   name=f"sc2_{b}_{cq}")
                    nc.sync.dma_start(
                        out=sc2[:, 0:8],
                        in_=scr[:].rearrange("p (a c) -> p a c", a=128))
                    rc = scp.tile([128, 8], F16, tag="rc",
                                  name=f"rc{b}_{cq}")
                    with nc.allow_low_precision(reason="4/sum fits fp16"):
                        nc.vector.reciprocal(rc[:], sc2[:, 0:8])
                    dst_ap = bass.AP(
                        tensor=rrec[:].tensor,
                        offset=(b * QC + cq) * 1024,
                        ap=[[8, 128], [1, 8]],
                    )
                    nc.sync.dma_start(out=dst_ap, in_=rc[:])

                def cols(tk):
                    j = tk - 4 * cq
                    return 2 * (512 - 128 * j) if j > 0 else 1024

                thunks = [(cols(tk) + (cols(tk - 2) if tk >= 2 else 0),
                           lambda tk=tk: step(tk)) for tk in range(ntk)]
                thunks.append((cols(ntk - 2) + cols(ntk - 1), flush))
                thunks.append((0, finish))
                return thunks

            def gen_norm(b, cq, onorm, late=False):
                """Broadcast 4/sum from DRAM and normalize onorm chunk.
                late=True keeps the endgame off gpsimd, whose software-DGE
                drain (~5.6us) would otherwise gate the epilogue."""
                def go():
                    cqs = slice(cq * 512, cq * 512 + 512)
                    bc = bcp.tile([128, 512], F16, tag="bc",
                                  name=f"bc{b}_{cq}")
                    for h in (0, 1):
                        src = bass.AP(
                            tensor=rrec[:].tensor,
                            offset=(b * QC + cq) * 1024 + h * 512,
                            ap=[[0, 64], [1, 512]],
                        )
                        if h == 0:
                            eng = nc.sync
                        else:
                            eng = nc.scalar if late else nc.gpsimd
                        eng.dma_start(
                            out=bc[h * 64:(h + 1) * 64, :], in_=src)
                    nc.vector.tensor_mul(onorm[:, cqs], onorm[:, cqs],
                                         bc[:])
                return [(0, go)]

            def gen_wout(b, cq, onorm, act_cast=False, late=False):
                def wstep(qi):
                    ysb = ysp.tile([128, DM], F16, tag="y",
                                   name=f"y{b}_{qi}")
                    for nh in range(2):
                        yps = psf.tile([128, 512], F32, tag="bank",
                                       name=f"yp{b}_{qi}_{nh}")
                        nc.tensor.matmul(
                            yps[:],
                            onorm[:, qi * 128:(qi + 1) * 128],
                            wout_t[:, nh * 512:(nh + 1) * 512],
                            start=True, stop=True,
                        )
                        cs = slice(nh * 512, nh * 512 + 512)
                        if act_cast and nh == 0:
                            nc.scalar.mul(ysb[:, cs], yps[:], 1.0)
                        else:
                            nc.vector.tensor_copy(ysb[:, cs], yps[:])
                    if act_cast or late:
                        # endgame groups: avoid gpsimd (slow software-DGE
                        # drain) and scalar (exp stream still running)
                        eng = nc.sync
                    else:
                        eng = nc.gpsimd if qi % 2 else nc.sync
                    eng.dma_start(
                        out=y[b, qi * 128:(qi + 1) * 128, :], in_=ysb)
                return [(1024, lambda qi=qi: wstep(qi))
                        for qi in range(4 * cq, 4 * cq + 4)]

            def interleave(primary, fillers):
                """Emit primary thunks in order, distributing fillers so
                filler PE-cycles track primary PE-cycles proportionally."""
                pt_ = sum(c for c, _ in primary) or 1
                ft = sum(c for c, _ in fillers)
                fi = 0
                pacc = 0
                facc = 0
                for c, t in primary:
                    t()
                    pacc += c
                    while fi < len(fillers) and facc * pt_ <= pacc * ft:
                        fc, f = fillers[fi]
                        f()
                        facc += max(fc, 1)
                        fi += 1
                while fi < len(fillers):
                    fillers[fi][1]()
                    fi += 1

            # ---- schedule ----
            load_xt(0)
            cur = {}
            for _, t in gen_proj(0, cur):
                t()
            for _, t in gen_vtrans(0, cur["vt"], vaugs[0], range(KT)):
                t()
            # xt1 issues from gpsimd only, gated behind batch 0's chunk-1
            # projection landing in SBUF, so batch 0's xt + weights have
            # the DMA engines to themselves during startup.
            nc.gpsimd.tensor_copy(dumt[:], cur["qt"][0:1, 1023:1024])
            load_xt(1, eng_all=nc.gpsimd)

            nxt = {}
            pend_w = []     # pending wout filler groups
            prev_norm = []  # norm(b, 3) carried to batch b+1 chunk 0
            prev = {}   # previous batch's leftover vtrans thunks
            for b in range(B):
                onorm = onp.tile([128, SEQ], F16, tag="onorm",
                                 name=f"on{b}")
                last = (b + 1 >= B)
                if not last:
                    nxt = {}
                    proj_next = gen_proj(b + 1, nxt)
                else:
                    proj_next = []
                order = [3, 2, 1, 0] if last else [0, 1, 2, 3]
                for idx, cq in enumerate(order):
                    att = gen_attention(b, cq, cur["qt"], cur["kt"],
                                        onorm, vaugs[b % 2])
                    # norm of the previous processed chunk goes first
                    # (cheap, off-PE, long-latency broadcast chain); wout
                    # groups run one full phase after their norm was issued
                    # so the chain never stalls the PE.
                    fill = []
                    if idx > 0:
                        fill += gen_norm(b, order[idx - 1], onorm,
                                         late=(last and idx == 3))
                    elif prev_norm:
                        fill += prev_norm.pop()
                    if not last:
                        fill += proj_next[6 * cq: 6 * cq + 6]
                        # vtrans trails its proj chunk by TWO phases so the
                        # transpose never waits on a lagging DVE proj copy
                        if cq >= 2:
                            lo = 4 * (cq - 2)
                            # batch 3 runs reversed; all 16 V transposes
                            # must land before its first (biggest) chunk
                            hi = 16 if (b == 2 and cq == 3) else lo + 4
                            fill += gen_vtrans(b + 1, nxt["vt"],
                                               vaugs[(b + 1) % 2],
                                               range(lo, hi))
                    if b == 0 and cq == 2:
                        fill.append((0, lambda: load_xt(2,
                                                        eng_all=nc.gpsimd)))
                    if b == 1 and cq == 2:
                        fill.append((0, lambda: load_xt(3,
                                                        eng_all=nc.gpsimd)))
                    if idx == 0:
                        fill += prev.pop("vtrans", [])
                    # wout one phase after its norm, at the end of the fill
                    # list (interleave emits fillers progressively).
                    if idx < 2 and pend_w:
                        fill += pend_w.pop(0)
                    if idx >= 2:
                        fill += gen_wout(b, order[idx - 2], onorm,
                                         late=(last and idx == 3))
                    interleave(att, fill)
                if not last:
                    pend_w.append(gen_wout(b, 2, onorm))
                    pend_w.append(gen_wout(b, 3, onorm))
                    prev_norm.append(gen_norm(b, 3, onorm))
                    if b < 2:
                        prev = {
                            "vtrans": gen_vtrans(b + 1, nxt["vt"],
                                                 vaugs[(b + 1) % 2],
                                                 range(8, 16)),
                        }
                    cur = nxt
                else:
                    tail = gen_norm(b, 0, onorm, late=True)
                    tail += gen_wout(b, 1, onorm, act_cast=True)
                    tail += gen_wout(b, 0, onorm, act_cast=True)
                    for _, t in tail:
                        t()

    return _patch_nc(nc)


def _prepare_in_maps(residual_stream, weight_query, weight_key, weight_value,
                     weight_out):
    x = np.asarray(residual_stream, np.float32)
    # xt[b, p, cq, m, q] = x[b, cq*512+q, m*128+p]
    xt = np.ascontiguousarray(
        x.reshape(B, QC, 512, MT, 128).transpose(0, 4, 1, 3, 2)
    ).astype(np.float16)
    tri = np.zeros((128, 128), np.float32)
    k_i = np.arange(128)[:, None]
    q_i = np.arange(128)[None, :]
    tri[:] = (q_i >= k_i).astype(np.float32)
    ident = np.eye(128, dtype=np.float32)
    in_maps = []
    for c in range(NCORES):
        w = np.empty((128, MT, 3, 128), np.float32)
        for proj, wt in ((0, weight_query), (1, weight_key),
                         (2, weight_value)):
            # per-core head pair -> [1024, 128] col-block, then m-tiled
            wc = np.asarray(wt[HPC * c:HPC * (c + 1)], np.float32)
            wc = wc.transpose(1, 0, 2).reshape(DM, HPC * DH)
            w[:, :, proj, :] = wc.reshape(MT, 128, HPC * DH).transpose(1, 0, 2)
        wo = np.ascontiguousarray(
            np.asarray(weight_out, np.float32)[128 * c:128 * (c + 1), :]
        )
        in_maps.append({
            "xt": xt,
            "w_all": np.ascontiguousarray(w).astype(np.float16),
            "wout": wo.astype(np.float16),
            "tri": tri.astype(np.float16),
            "ident": ident.astype(np.float16),
        })
    return in_maps


def kernel(residual_stream, weight_query, weight_key, weight_value,
           weight_out, trace=False):
    from concourse.bass_utils import run_bass_kernel_spmd

    if "nc" not in _CACHE:
        _CACHE["nc"] = _build_nc()
    nc = _CACHE["nc"]

    in_maps = _prepare_in_maps(
        residual_stream, weight_query, weight_key, weight_value, weight_out
    )
    res = run_bass_kernel_spmd(
        nc, in_maps, list(range(NCORES)), trace=trace
    )
    _CACHE["last_result"] = res
    out = np.zeros((B, SEQ, DM), np.float32)
    for c in range(NCORES):
        out += np.asarray(res.results[c]["y"], np.float32)
    return out


# revision 67
# speedup vs baseline: 1.0412x; 1.0412x over previous
"""Causal multi-head attention on 8 Trainium2 NeuronCores — v3.

Problem: residual_stream [4, 2048, 1024] fp32, per-head QKV weights
[16, 1024, 64], output projection [1024, 1024]; causal softmax attention.

Sharding: tensor-parallel over heads — core c computes heads (2c, 2c+1)
for all 4 batches, applies the matching 128-row slice of weight_out, and
returns a full-shape fp16 partial output; the host sums the 8 partials.

v3 changes vs v2 (344us):
  * Startup: warmup matmuls run on a memset tile (no DMA dependency),
    weights DMA'd per-projection so the first QKV matmul starts as soon
    as w_q + xt chunk 0 land (v2 waited ~21us for the full weight+xt
    race to drain).
  * Normalize chain shortened: V_aug ones column holds 0.25 so the PV
    ones-row accumulates sum/4 directly; 4/sum comes from a single-pass
    reciprocal_approx_fast straight out of PSUM (v2: two tensor_scalar
    evacuations + DMA gather + full-rate reciprocal).  One broadcast
    multiply [128,512] instead of two [64,512].
  * Uniform 1-phase wout deferral (norm of chunk c early in chunk c+1's
    fill, wout late in the same fill), instead of v2's 2-phase chains
    that piled 6 wout groups into batch 3.
  * Batch 3 runs its chunks in reverse (3,2,1,0) so the serial tail is
    the smallest chunk; tail wout casts split ACT/DVE; V transposes for
    batch 3 complete during batch 2.
"""
import sys
import types

sys.path.insert(0, "/opt/trn_rl_repo")

import ml_dtypes
import numpy as np

import concourse.bass as bass
import concourse.tile as tile
from concourse import mybir

F32 = mybir.dt.float32
F16 = mybir.dt.float16

B = 4
SEQ = 2048
DM = 1024
DH = 64
NH = 16
NCORES = 8
HPC = NH // NCORES          # heads per core = 2
MT = DM // 128              # m-tiles = 8
KT = SEQ // 128             # k-tiles = 16
QC = SEQ // 512             # q-chunks of 512 = 4
EXP_BIAS = -6.0             # exp(s/8 - 6): softmax-shift, keeps P in fp16
OSC = 0.25                  # O evacuated as O/4; ones col = 1/4 so the
                            # sums row holds sum/4 and recip gives 4/sum

_CACHE = {}


def _split_waits(d, max_waits=1):
    # This walrus build allows a single sync-wait on several instruction
    # encodings (CTRL Drain, fused-LDW f32 Matmult). Hoist excess waits
    # onto same-engine NoOp carriers directly in the BIR JSON.
    for fn in d.get("functions", []):
        for blk in fn.get("blocks", []):
            out = []
            for inst in blk.get("instructions", []):
                si = inst.get("sync_info") or {}
                waits = si.get("on_wait") or []
                if len(waits) > max_waits:
                    extra = waits[: len(waits) - max_waits]
                    rest = waits[len(waits) - max_waits:]
                    for i, w in enumerate(extra):
                        out.append({
                            "name": f"{inst['name']}_sw{i}",
                            "opcode": "NoOp",
                            "engine": inst["engine"],
                            "ins": [],
                            "outs": [],
                            "sync_info": {"on_update": [], "on_wait": [w]},
                        })
                    inst = dict(inst)
                    inst["sync_info"] = {
                        "on_update": list(si.get("on_update") or []),
                        "on_wait": rest,
                    }
                out.append(inst)
            blk["instructions"] = out
    return d


def _patch_nc(nc):
    import orjson

    def to_json_bytes(self):
        return orjson.dumps(
            _split_waits(orjson.loads(mybir.module_to_json_bytes(self.m)))
        )

    nc.to_json_bytes = types.MethodType(to_json_bytes, nc)
    return nc


def _build_nc():
    nc = bass.Bass()

    xt = nc.dram_tensor("xt", [B, 128, QC, MT, 512], F16,
                    kind="ExternalInput")
    w_all = nc.dram_tensor("w_all", [128, MT, 3, 128], F16,
                           kind="ExternalInput")
    wout = nc.dram_tensor("wout", [128, DM], F16, kind="ExternalInput")
    tri = nc.dram_tensor("tri", [128, 128], F16, kind="ExternalInput")
    ident = nc.dram_tensor("ident", [128, 128], F16, kind="ExternalInput")
    y = nc.dram_tensor("y", [B, SEQ, DM], F16, kind="ExternalOutput")
    rrec = nc.dram_tensor("rrec", [B, QC, 1024], F16)   # 4/sum bounce

    with tile.TileContext(nc) as tc:
        with (
            tc.tile_pool(name="const", bufs=1) as const,
            tc.tile_pool(name="xtp", bufs=2) as xtp,
            tc.tile_pool(name="qp", bufs=2) as qp,
            tc.tile_pool(name="kp", bufs=2) as kp,
            tc.tile_pool(name="vp", bufs=2) as vp,
            tc.tile_pool(name="ptp", bufs=8) as ptp,
            tc.tile_pool(name="onp", bufs=2) as onp,
            tc.tile_pool(name="ysp", bufs=6) as ysp,
            tc.tile_pool(name="scp", bufs=2) as scp,
            tc.tile_pool(name="bcp", bufs=2) as bcp,
            tc.tile_pool(name="pss", bufs=2, space="PSUM") as pss,
            tc.tile_pool(name="pso", bufs=2, space="PSUM") as pso,
            tc.tile_pool(name="psf", bufs=2, space="PSUM") as psf,
        ):
            wrm = const.tile([128, 512], F16, tag="wrm")
            nc.gpsimd.memset(wrm[:], 0.001)
            dumt = const.tile([1, 1], F16, tag="dumt")

            w_t = const.tile([128, MT, 3, 128], F16, tag="w")
            nc.scalar.dma_start(out=w_t[:, 0:4], in_=w_all[:, 0:4])
            nc.scalar.dma_start(out=w_t[:, 4:8], in_=w_all[:, 4:8])
            wout_t = const.tile([128, DM], F16, tag="wout")
            nc.scalar.dma_start(out=wout_t, in_=wout[:])
            tri_t = const.tile([128, 128], F16, tag="tri")
            nc.scalar.dma_start(out=tri_t, in_=tri[:])
            ident_t = const.tile([128, 128], F16, tag="ident")
            nc.scalar.dma_start(out=ident_t, in_=ident[:])
            bias_t = const.tile([128, 1], F32, tag="bias")
            nc.gpsimd.memset(bias_t[:], EXP_BIAS)

            # V_aug double-buffered (batch parity); ones columns hold 1/4
            # so the PV ones-row accumulates sum/4.
            vaug0 = const.tile([128, KT, HPC, 65], F16, tag="vaug0")
            vaug1 = const.tile([128, KT, HPC, 65], F16, tag="vaug1")
            vaugs = [vaug0, vaug1]
            nc.gpsimd.memset(vaug0[:, :, :, 64:65], OSC)
            nc.gpsimd.memset(vaug1[:, :, :, 64:65], OSC)

            warm = psf.tile([128, 512], F32, tag="bank", name="warm")
            for i in range(30):
                nc.tensor.matmul(
                    warm[:],
                    wrm[:, 0:128],
                    wrm[:],
                    start=True, stop=True,
                )

            xts = {}

            def load_xt(b, eng_all=None):
                xt_t = xtp.tile([128, QC, MT, 512], F16, tag="xt",
                                name=f"xt{b}")
                xts[b] = xt_t
                for cq in range(QC):
                    if b == 0 and cq == 0:
                        # first chunk split across both queues: the very
                        # first projection only waits for half the bytes
                        nc.sync.dma_start(out=xt_t[:, 0, 0:4],
                                          in_=xt[b, :, 0, 0:4])
                        nc.gpsimd.dma_start(out=xt_t[:, 0, 4:8],
                                            in_=xt[b, :, 0, 4:8])
                        continue
                    eng = eng_all or (nc.gpsimd if cq % 2 else nc.sync)
                    eng.dma_start(out=xt_t[:, cq], in_=xt[b, :, cq])

            def gen_proj(b, out):
                """QKV projections for batch b as PE thunks (fine-grained:
                one thunk per (chunk, proj, m-half)). xts[b] is looked up
                lazily so load_xt(b) may be scheduled after gen time."""
                qt = qp.tile([128, SEQ], F16, tag="qt", name=f"qt{b}")
                kt_t = kp.tile([128, SEQ], F16, tag="kt", name=f"kt{b}")
                vt = vp.tile([128, SEQ], F16, tag="vt", name=f"vt{b}")
                out["qt"], out["kt"], out["vt"] = qt, kt_t, vt
                state = {}
                thunks = []

                def step(cq, proj, half):
                    xt_t = xts[b]
                    cqs = slice(cq * 512, cq * 512 + 512)
                    if half == 0:
                        state["ps"] = psf.tile([128, 512], F32, tag="bank",
                                               name=f"pj{b}_{cq}_{proj}")
                    ps = state["ps"]
                    for m in range(half * 4, half * 4 + 4):
                        nc.tensor.matmul(
                            ps[:],
                            w_t[:, m, proj, :],
                            xt_t[:, cq, m, :],
                            start=(m == 0),
                            stop=(m == MT - 1),
                        )
                    if half == 1:
                        if proj == 0:
                            nc.vector.tensor_copy(qt[:, cqs], ps[:])
                        elif proj == 1:
                            nc.vector.tensor_copy(kt_t[:, cqs], ps[:])
                        else:
                            nc.vector.tensor_copy(vt[:, cqs], ps[:])

                for cq in range(QC):
                    for proj in range(3):
                        for half in range(2):
                            thunks.append(
                                (2048,
                                 lambda cq=cq, proj=proj, half=half:
                                 step(cq, proj, half)))
                return thunks

            def gen_vtrans(b, vt, vaug, tks):
                """V^T -> V_aug transposes as PE thunks for given k-tiles."""
                def tstep(tk):
                    pt_ps = psf.tile([128, 128], F16, tag="bank",
                                     name=f"tp{b}_{tk}")
                    nc.tensor.transpose(
                        pt_ps[:], vt[:, tk * 128:(tk + 1) * 128], ident_t[:]
                    )
                    nc.vector.tensor_copy(vaug[:, tk, :, 0:64], pt_ps[:])
                return [(128, lambda tk=tk: tstep(tk)) for tk in tks]

            def gen_attention(b, cq, qt, kt_t, onorm, vaug):
                """Attention thunks for one (batch, chunk): one per k-tile
                plus trailing pipeline flush + chunk-finish."""
                ntk = 4 * cq + 4
                st = {"ops": None, "pend": []}
                cq0 = cq * 512

                def make_pt(tk):
                    j = tk - 4 * cq
                    off = 128 * j if j > 0 else 0
                    sdbl = pss.tile([128, 1024], F32, tag="bank",
                                    name=f"s{b}_{cq}_{tk}")
                    tks = slice(tk * 128, tk * 128 + 128)
                    for h in (0, 1):
                        hs = slice(h * 64, h * 64 + 64)
                        nc.tensor.matmul(
                            sdbl[:, h * 512 + off:(h + 1) * 512],
                            kt_t[hs, tks],
                            qt[hs, cq0 + off:cq0 + 512],
                            start=True, stop=True,
                        )
                    pt = ptp.tile([128, 1024], F16, tag="pt",
                                  name=f"pt{b}_{cq}_{tk}")
                    if j <= 0:
                        nc.scalar.activation(
                            pt[:], sdbl[:],
                            mybir.ActivationFunctionType.Exp,
                            bias=bias_t[:], scale=0.125,
                        )
                    else:
                        sr = sdbl[:].rearrange("p (h q) -> p h q",
                                               h=2)[:, :, off:512]
                        pr = pt[:].rearrange("p (h q) -> p h q",
                                             h=2)[:, :, off:512]
                        nc.scalar.activation(
                            pr, sr,
                            mybir.ActivationFunctionType.Exp,
                            bias=bias_t[:], scale=0.125,
                        )
                    if j >= 0:
                        pm = pt[:].rearrange(
                            "p (h q) -> p h q",
                            h=2)[:, :, 128 * j:128 * j + 128]
                        ta = tri_t[:]
                        tb = bass.AP(tensor=ta.tensor, offset=ta.offset,
                                     ap=[ta.ap[0], [0, 2], ta.ap[1]])
                        nc.vector.tensor_mul(pm, pm, tb)
                    return off, pt

                def pv_step(tk, off, pt):
                    for h in (0, 1):
                        nc.tensor.matmul(
                            st["ops"][h][:, off:512],
                            vaug[:, tk, h, :],
                            pt[:, h * 512 + off:(h + 1) * 512],
                            start=(tk == 0), stop=(tk == ntk - 1),
                        )

                def step(tk):
                    if tk == 0:
                        st["ops"] = {
                            h: pso.tile([65, 512], F32, tag="bank",
                                        name=f"o{b}_{cq}_{h}")
                            for h in (0, 1)
                        }
                    st["pend"].append((tk, make_pt(tk)))
                    if len(st["pend"]) > 2:
                        t0, (off0, p0) = st["pend"].pop(0)
                        pv_step(t0, off0, p0)

                def flush():
                    while st["pend"]:
                        t0, (off0, p0) = st["pend"].pop(0)
                        pv_step(t0, off0, p0)

                def finish():
                    cqs = slice(cq0, cq0 + 512)
                    ops = st["ops"]
                    # O/4 -> onorm fp16 (h0 on ACT, h1 on DVE); sums rows
                    # (already sum/4 via the 0.25 ones col) evacuate on ACT,
                    # then DMA-gather to [128,8] so the reciprocal runs wide
                    # (a [1,512] DVE op is one serial lane, ~3.3us).
                    nc.scalar.mul(onorm[0:64, cqs], ops[0][0:64, :], OSC)
                    nc.vector.tensor_scalar_mul(
                        onorm[64:128, cqs], ops[1][0:64, :], OSC)
                    scr = scp.tile([1, 1024], F32, tag="scr",
                                   name=f"scr{b}_{cq}")
                    nc.vector.tensor_copy(scr[:, 0:512], ops[0][64:65, :])
                    nc.vector.tensor_copy(scr[:, 512:1024],
                                          ops[1][64:65, :])
                    sc2 = scp.tile([128, 16], F32, tag="sc2",
                                   name=f"sc2_{b}_{cq}")
                    nc.sync.dma_start(
                        out=sc2[:, 0:8],
                        in_=scr[:].rearrange("p (a c) -> p a c", a=128))
                    rc = scp.tile([128, 8], F16, tag="rc",
                                  name=f"rc{b}_{cq}")
                    with nc.allow_low_precision(reason="4/sum fits fp16"):
                        nc.vector.reciprocal(rc[:], sc2[:, 0:8])
                    dst_ap = bass.AP(
                        tensor=rrec[:].tensor,
                        offset=(b * QC + cq) * 1024,
                        ap=[[8, 128], [1, 8]],
                    )
                    nc.sync.dma_start(out=dst_ap, in_=rc[:])

                def cols(tk):
                    j = tk - 4 * cq
                    return 2 * (512 - 128 * j) if j > 0 else 1024

                thunks = [(cols(tk) + (cols(tk - 2) if tk >= 2 else 0),
                           lambda tk=tk: step(tk)) for tk in range(ntk)]
                thunks.append((cols(ntk - 2) + cols(ntk - 1), flush))
                thunks.append((0, finish))
                return thunks

            def gen_norm(b, cq, onorm, late=False):
                """Broadcast 4/sum from DRAM and normalize onorm chunk.
                late=True keeps the endgame off gpsimd, whose software-DGE
                drain (~5.6us) would otherwise gate the epilogue."""
                def go():
                    cqs = slice(cq * 512, cq * 512 + 512)
                    bc = bcp.tile([128, 512], F16, tag="bc",
                                  name=f"bc{b}_{cq}")
                    for h in (0, 1):
                        src = bass.AP(
                            tensor=rrec[:].tensor,
                            offset=(b * QC + cq) * 1024 + h * 512,
                            ap=[[0, 64], [1, 512]],
                        )
                        if h == 0:
                            eng = nc.sync
                        else:
                            eng = nc.scalar if late else nc.gpsimd
                        eng.dma_start(
                            out=bc[h * 64:(h + 1) * 64, :], in_=src)
                    nc.vector.tensor_mul(onorm[:, cqs], onorm[:, cqs],
                                         bc[:])
                return [(0, go)]

            def gen_wout(b, cq, onorm, act_cast=False, late=False):
                def wstep(qi):
                    ysb = ysp.tile([128, DM], F16, tag="y",
                                   name=f"y{b}_{qi}")
                    for nh in range(2):
                        yps = psf.tile([128, 512], F32, tag="bank",
                                       name=f"yp{b}_{qi}_{nh}")
                        nc.tensor.matmul(
                            yps[:],
                            onorm[:, qi * 128:(qi + 1) * 128],
                            wout_t[:, nh * 512:(nh + 1) * 512],
                            start=True, stop=True,
                        )
                        cs = slice(nh * 512, nh * 512 + 512)
                        if act_cast and nh == 0:
                            nc.scalar.mul(ysb[:, cs], yps[:], 1.0)
                        else:
                            nc.vector.tensor_copy(ysb[:, cs], yps[:])
                    if act_cast or late:
                        # endgame groups: avoid gpsimd (slow software-DGE
                        # drain) and scalar (exp stream still running)
                        eng = nc.sync
                    else:
                        eng = nc.gpsimd if qi % 2 else nc.sync
                    eng.dma_start(
                        out=y[b, qi * 128:(qi + 1) * 128, :], in_=ysb)
                return [(1024, lambda qi=qi: wstep(qi))
                        for qi in range(4 * cq, 4 * cq + 4)]

            def interleave(primary, fillers):
                """Emit primary thunks in order, distributing fillers so
                filler PE-cycles track primary PE-cycles proportionally."""
                pt_ = sum(c for c, _ in primary) or 1
                ft = sum(c for c, _ in fillers)
                fi = 0
                pacc = 0
                facc = 0
                for c, t in primary:
                    t()
                    pacc += c
                    while fi < len(fillers) and facc * pt_ <= pacc * ft:
                        fc, f = fillers[fi]
                        f()
                        facc += max(fc, 1)
                        fi += 1
                while fi < len(fillers):
                    fillers[fi][1]()
                    fi += 1

            # ---- schedule ----
            load_xt(0)
            cur = {}
            for _, t in gen_proj(0, cur):
                t()
            for _, t in gen_vtrans(0, cur["vt"], vaugs[0], range(KT)):
                t()
            # xt1 issues from gpsimd only, gated behind batch 0's chunk-1
            # projection landing in SBUF, so batch 0's xt + weights have
            # the DMA engines to themselves during startup.
            nc.gpsimd.tensor_copy(dumt[:], cur["qt"][0:1, 1023:1024])
            load_xt(1, eng_all=nc.gpsimd)

            nxt = {}
            pend_w = []     # pending wout filler groups
            prev_norm = []  # norm(b, 3) carried to batch b+1 chunk 0
            prev = {}   # previous batch's leftover vtrans thunks
            for b in range(B):
                onorm = onp.tile([128, SEQ], F16, tag="onorm",
                                 name=f"on{b}")
                last = (b + 1 >= B)
                if not last:
                    nxt = {}
                    proj_next = gen_proj(b + 1, nxt)
                else:
                    proj_next = []
                order = [3, 2, 1, 0] if last else [0, 1, 2, 3]
                for idx, cq in enumerate(order):
                    att = gen_attention(b, cq, cur["qt"], cur["kt"],
                                        onorm, vaugs[b % 2])
                    # norm of the previous processed chunk goes first
                    # (cheap, off-PE, long-latency broadcast chain); wout
                    # groups run one full phase after their norm was issued
                    # so the chain never stalls the PE.
                    fill = []
                    if idx > 0:
                        fill += gen_norm(b, order[idx - 1], onorm,
                                         late=(last and idx == 3))
                    elif prev_norm:
                        fill += prev_norm.pop()
                    if not last:
                        fill += proj_next[6 * cq: 6 * cq + 6]
                        # vtrans trails its proj chunk by TWO phases so the
                        # transpose never waits on a lagging DVE proj copy
                        if cq >= 2:
                            lo = 4 * (cq - 2)
                            # batch 3 runs reversed; all 16 V transposes
                            # must land before its first (biggest) chunk
                            hi = 16 if (b == 2 and cq == 3) else lo + 4
                            fill += gen_vtrans(b + 1, nxt["vt"],
                                               vaugs[(b + 1) % 2],
                                               range(lo, hi))
                    if b == 0 and cq == 2:
                        fill.append((0, lambda: load_xt(2,
                                                        eng_all=nc.gpsimd)))
                    if b == 1 and cq == 2:
                        fill.append((0, lambda: load_xt(3,
                                                        eng_all=nc.gpsimd)))
                    if idx == 0:
                        fill += prev.pop("vtrans", [])
                    # wout one phase after its norm, at the end of the fill
                    # list (interleave emits fillers progressively).
                    if last:
                        # the biggest (first-processed, reversed) chunk
                        # absorbs both carried groups; after that each
                        # chunk hosts exactly one group, one phase behind
                        # its norm, leaving a single group for the tail
                        if idx == 0:
                            while pend_w:
                                fill += pend_w.pop(0)
                        else:
                            fill += gen_wout(b, order[idx - 1], onorm,
                                             late=(idx == 3))
                    else:
                        if idx < 2 and pend_w:
                            fill += pend_w.pop(0)
                        if idx >= 2:
                            fill += gen_wout(b, order[idx - 2], onorm)
                    interleave(att, fill)
                if not last:
                    pend_w.append(gen_wout(b, 2, onorm))
                    pend_w.append(gen_wout(b, 3, onorm))
                    prev_norm.append(gen_norm(b, 3, onorm))
                    if b < 2:
                        prev = {
                            "vtrans": gen_vtrans(b + 1, nxt["vt"],
                                                 vaugs[(b + 1) % 2],
                                                 range(8, 16)),
                        }
                    cur = nxt
                else:
                    tail = gen_norm(b, 0, onorm, late=True)
                    tail += gen_wout(b, 0, onorm, act_cast=True)
                    for _, t in tail:
                        t()

    return _patch_nc(nc)


def _prepare_in_maps(residual_stream, weight_query, weight_key, weight_value,
                     weight_out):
    x = np.asarray(residual_stream, np.float32)
    # xt[b, p, cq, m, q] = x[b, cq*512+q, m*128+p]
    xt = np.ascontiguousarray(
        x.reshape(B, QC, 512, MT, 128).transpose(0, 4, 1, 3, 2)
    ).astype(np.float16)
    tri = np.zeros((128, 128), np.float32)
    k_i = np.arange(128)[:, None]
    q_i = np.arange(128)[None, :]
    tri[:] = (q_i >= k_i).astype(np.float32)
    ident = np.eye(128, dtype=np.float32)
    in_maps = []
    for c in range(NCORES):
        w = np.empty((128, MT, 3, 128), np.float32)
        for proj, wt in ((0, weight_query), (1, weight_key),
                         (2, weight_value)):
            # per-core head pair -> [1024, 128] col-block, then m-tiled
            wc = np.asarray(wt[HPC * c:HPC * (c + 1)], np.float32)
            wc = wc.transpose(1, 0, 2).reshape(DM, HPC * DH)
            w[:, :, proj, :] = wc.reshape(MT, 128, HPC * DH).transpose(1, 0, 2)
        wo = np.ascontiguousarray(
            np.asarray(weight_out, np.float32)[128 * c:128 * (c + 1), :]
        )
        in_maps.append({
            "xt": xt,
            "w_all": np.ascontiguousarray(w).astype(np.float16),
            "wout": wo.astype(np.float16),
            "tri": tri.astype(np.float16),
            "ident": ident.astype(np.float16),
        })
    return in_maps


def kernel(residual_stream, weight_query, weight_key, weight_value,
           weight_out, trace=False):
    from concourse.bass_utils import run_bass_kernel_spmd

    if "nc" not in _CACHE:
        _CACHE["nc"] = _build_nc()
    nc = _CACHE["nc"]

    in_maps = _prepare_in_maps(
        residual_stream, weight_query, weight_key, weight_value, weight_out
    )
    res = run_bass_kernel_spmd(
        nc, in_maps, list(range(NCORES)), trace=trace
    )
    _CACHE["last_result"] = res
    out = np.zeros((B, SEQ, DM), np.float32)
    for c in range(NCORES):
        out += np.asarray(res.results[c]["y"], np.float32)
    return out
